# revision 4
# baseline (speedup 1.0000x reference)
"""CQAttention trilinear-similarity kernel for 8 Trainium2 NeuronCores.

Shapes (full problem): C [16,1024,512], Q [16,256,512] -> out [16,1024,2048].
Sharding: pure batch parallel, 2 batch elements per core, SPMD NEFF.

Math (per batch element), faithful to the reference modulo the max-shift:
  S = (C*w4mlu) @ Q^T + C@w4C + (Q@w4Q)^T + bias          [1024, 256]
  S1 = masked_softmax(S, Qmask, axis=Lq)
  S2 = masked_softmax(S, Cmask, axis=Lc)
  A  = S1 @ Q;  D = S2^T @ C;  Bt = S1 @ D
  out = [C | A | C*A | C*Bt]

Because the reference clips S to [-15,15] *before* exp, exp never overflows,
so the max-subtraction is skipped (error <= 1e-6 relative, dominated by the
reference's own +1e-6 denominator epsilon).  Multiplicative {0,1} masks are
replaced by an additive -60 inside the exp argument (exp(-45) ~ 3e-20 ~ 0),
which lets the mask ride in the scalar-engine activation bias for free.
"""

import os
import sys

for _p in ("/opt/trn_rl_repo", "/root/.axon_site/_ro/trn_rl_repo"):
    if os.path.isdir(_p) and _p not in sys.path:
        sys.path.insert(0, _p)

import numpy as np

import concourse.bacc as bacc
import concourse.mybir as mybir
import concourse.tile as tile
from concourse.bass_utils import run_bass_kernel_spmd
from concourse.masks import make_identity

F32 = mybir.dt.float32
ALU = mybir.AluOpType
ACTF = mybir.ActivationFunctionType

B, LC, LQ, D = 16, 1024, 256, 512
NCORES = 8
BPC = B // NCORES          # batch elements per core
NJ = LC // 128             # 8 Lc tiles
NM = LQ // 128             # 2 Lq tiles
NK = D // 128              # 4 d chunks
MASK_NEG = 60.0


def emit(tc, ins, out_d):
    nc = tc.nc
    ct_d = ins["CT"]
    cn_d = ins["Cn"]
    qn_d = ins["Qn"]
    qt_d = ins["QT"]
    qm_d = ins["qmadd"]
    cm_d = ins["cmadd"]
    wmlu_d = ins["wmlu"]
    wc_d = ins["wc"]
    wq_d = ins["wq"]
    bias_d = ins["biascol"]
    if True:
        with tc.tile_pool(name="consts", bufs=1) as consts, \
             tc.tile_pool(name="io", bufs=2) as io, \
             tc.tile_pool(name="io1", bufs=1) as io1, \
             tc.tile_pool(name="mid", bufs=2) as mid, \
             tc.tile_pool(name="stgp", bufs=3) as stgp, \
             tc.tile_pool(name="aux", bufs=2) as aux, \
             tc.tile_pool(name="psb", bufs=6, space="PSUM") as psb, \
             tc.tile_pool(name="pss", bufs=2, space="PSUM") as pss:

            ident = consts.tile([128, 128], F32)
            make_identity(nc, ident[:])
            onesc = consts.tile([128, 1], F32)
            nc.vector.memset(onesc[:], 1.0)
            wmlu = consts.tile([128, NK], F32)
            wc = consts.tile([128, NK], F32)
            wq = consts.tile([128, NK], F32)
            biascol = consts.tile([128, 1], F32)
            nc.sync.dma_start(wmlu[:], wmlu_d[:])
            nc.sync.dma_start(wc[:], wc_d[:])
            nc.sync.dma_start(wq[:], wq_d[:])
            nc.sync.dma_start(biascol[:], bias_d[:])

            for b in range(BPC):
                # ---- input loads -------------------------------------------------
                ct = io1.tile([128, NK, LC], F32, tag="ct")
                nc.sync.dma_start(ct[:], ct_d[b].rearrange("k p n -> p k n"))
                cn = io.tile([128, NJ, D], F32, tag="cn")
                nc.sync.dma_start(cn[:], cn_d[b].rearrange("j p n -> p j n"))
                qn = io.tile([128, NM, D], F32, tag="qn")
                nc.sync.dma_start(qn[:], qn_d[b].rearrange("m p n -> p m n"))
                qt = io.tile([128, NK, LQ], F32, tag="qt")
                nc.sync.dma_start(qt[:], qt_d[b].rearrange("k p n -> p k n"))
                qm = io.tile([128, NM], F32, tag="qm")
                nc.sync.dma_start(qm[:], qm_d[b])
                cm = io.tile([128, NJ], F32, tag="cm")
                nc.sync.dma_start(cm[:], cm_d[b])

                # ---- QTm = QT * w4mlu + w4C  (folds sub2 scaling and sub0) ------
                qtm = mid.tile([128, NK, LQ], F32, tag="qtm")
                for k in range(NK):
                    nc.vector.tensor_scalar(qtm[:, k], qt[:, k],
                                            wmlu[:, k:k + 1], wc[:, k:k + 1],
                                            ALU.mult, ALU.add)

                # ---- sub1 + bias, per Lq tile -----------------------------------
                s1b = mid.tile([128, NM], F32, tag="s1b")
                for m in range(NM):
                    ps_sub1 = pss.tile([128, 1], F32, tag="psmall")
                    for k in range(NK):
                        nc.tensor.matmul(ps_sub1[:], qt[:, k, m * 128:(m + 1) * 128],
                                         wq[:, k:k + 1],
                                         start=(k == 0), stop=(k == NK - 1))
                    nc.vector.tensor_tensor(s1b[:, m:m + 1], ps_sub1[:], biascol[:],
                                            ALU.add)

                # ---- S^T matmuls + clip chain + e1 = exp masked -----------------
                xc = []
                for m in range(NM):
                    x = mid.tile([128, LC], F32, tag=f"xc{m}")
                    for n in range(2):
                        ps_st = psb.tile([128, 512], F32, tag="pbig")
                        for k in range(NK):
                            nc.tensor.matmul(
                                ps_st[:],
                                qtm[:, k, m * 128:(m + 1) * 128],
                                ct[:, k, n * 512:(n + 1) * 512],
                                start=(k == 0), stop=(k == NK - 1))
                        # x = min(S^T + sub1 + bias, 15)
                        nc.vector.tensor_scalar(x[:, n * 512:(n + 1) * 512], ps_st[:],
                                                s1b[:, m:m + 1], 15.0,
                                                ALU.add, ALU.min)
                    nc.vector.tensor_scalar_max(x[:], x[:], -15.0)
                    xc.append(x)

                e1 = []
                for m in range(NM):
                    e = mid.tile([128, LC], F32, tag=f"e1{m}")
                    nc.scalar.activation(e[:], xc[m][:], ACTF.Exp,
                                         bias=qm[:, m:m + 1], scale=1.0)
                    e1.append(e)

                # ---- transpose x -> natural layout, e2 = exp masked -------------
                e2 = mid.tile([128, NJ, LQ], F32, tag="e2")
                for p in range(NJ // 2):
                    ps_xt = psb.tile([128, 2 * LQ], F32, tag="pbig")
                    for jj in range(2):
                        j = 2 * p + jj
                        for m in range(NM):
                            nc.tensor.transpose(
                                ps_xt[:, jj * LQ + m * 128: jj * LQ + (m + 1) * 128],
                                xc[m][:, j * 128:(j + 1) * 128], ident[:])
                    for jj in range(2):
                        j = 2 * p + jj
                        nc.scalar.activation(e2[:, j], ps_xt[:, jj * LQ:(jj + 1) * LQ],
                                             ACTF.Exp, bias=cm[:, j:j + 1], scale=1.0)

                # ---- s2 column sums -> r2 ---------------------------------------
                s2s = mid.tile([128, NM], F32, tag="s2s")
                for m in range(NM):
                    ps_s2 = pss.tile([128, 1], F32, tag="psmall")
                    for j in range(NJ):
                        nc.tensor.matmul(ps_s2[:], e2[:, j, m * 128:(m + 1) * 128],
                                         onesc[:],
                                         start=(j == 0), stop=(j == NJ - 1))
                    nc.vector.tensor_scalar_add(s2s[:, m:m + 1], ps_s2[:], 1e-6)
                r2 = mid.tile([128, NM], F32, tag="r2")
                nc.vector.reciprocal(r2[:], s2s[:])

                # ---- D = diag(r2) (e2^T @ C) ------------------------------------
                dD = mid.tile([128, NM, D], F32, tag="dD")
                for m in range(NM):
                    ps_d = psb.tile([128, D], F32, tag="pbig")
                    for j in range(NJ):
                        nc.tensor.matmul(ps_d[:], e2[:, j, m * 128:(m + 1) * 128],
                                         cn[:, j], start=(j == 0), stop=(j == NJ - 1))
                    nc.scalar.mul(dD[:, m], ps_d[:], r2[:, m:m + 1])

                # ---- A_raw, Bt_raw, s1 sums; normalize + combine + store --------
                for j in range(NJ):
                    ps_a = psb.tile([128, D], F32, tag="pbig")
                    ps_bt = psb.tile([128, D], F32, tag="pbig")
                    ps_s1 = pss.tile([128, 1], F32, tag="psmall")
                    for m in range(NM):
                        lhs = e1[m][:, j * 128:(j + 1) * 128]
                        nc.tensor.matmul(ps_a[:], lhs, qn[:, m],
                                         start=(m == 0), stop=(m == NM - 1))
                        nc.tensor.matmul(ps_bt[:], lhs, dD[:, m],
                                         start=(m == 0), stop=(m == NM - 1))
                        nc.tensor.matmul(ps_s1[:], lhs, onesc[:],
                                         start=(m == 0), stop=(m == NM - 1))
                    r1 = aux.tile([128, 1], F32, tag="r1")
                    nc.vector.tensor_scalar_add(r1[:], ps_s1[:], 1e-6)
                    nc.vector.reciprocal(r1[:], r1[:])

                    stg = stgp.tile([128, 3 * D], F32, tag="stg")
                    # A = A_raw * r1
                    nc.scalar.mul(stg[:, 0:D], ps_a[:], r1[:])
                    # C*A = (A_raw * r1) * C
                    nc.vector.scalar_tensor_tensor(stg[:, D:2 * D], ps_a[:], r1[:],
                                                   cn[:, j], ALU.mult, ALU.mult)
                    # Bt = Bt_raw * r1 (scratch), then C*Bt on gpsimd
                    btn = aux.tile([128, D], F32, tag="btn")
                    nc.scalar.mul(btn[:], ps_bt[:], r1[:])
                    nc.gpsimd.tensor_mul(stg[:, 2 * D:3 * D], btn[:], cn[:, j])

                    nc.scalar.dma_start(out_d[b, j][:, 0:D], cn[:, j])
                    nc.scalar.dma_start(out_d[b, j][:, D:4 * D], stg[:])


def build_program():
    nc = bacc.Bacc("TRN2", target_bir_lowering=False, debug=False,
                   enable_asserts=False, num_devices=NCORES)
    names_shapes = [
        ("CT", [BPC, NK, 128, LC]), ("Cn", [BPC, NJ, 128, D]),
        ("Qn", [BPC, NM, 128, D]), ("QT", [BPC, NK, 128, LQ]),
        ("qmadd", [BPC, 128, NM]), ("cmadd", [BPC, 128, NJ]),
        ("wmlu", [128, NK]), ("wc", [128, NK]), ("wq", [128, NK]),
        ("biascol", [128, 1]),
    ]
    ins = {n: nc.declare_dram_parameter(n, sh, F32, isOutput=False)
           for n, sh in names_shapes}
    out_d = nc.declare_dram_parameter("out", [BPC, NJ, 128, 4 * D], F32,
                                      isOutput=True)
    with tile.TileContext(nc) as tc:
        emit(tc, ins, out_d)
    nc.compile()
    return nc


_NC_CACHE = None


def _get_program():
    global _NC_CACHE
    if _NC_CACHE is None:
        _NC_CACHE = build_program()
    return _NC_CACHE


def make_in_maps(C, Q, Cmask, Qmask, w4C, w4Q, w4mlu, bias):
    C = np.ascontiguousarray(C, dtype=np.float32)
    Q = np.ascontiguousarray(Q, dtype=np.float32)
    qmadd = (Qmask.astype(np.float32) - 1.0) * MASK_NEG
    cmadd = (Cmask.astype(np.float32) - 1.0) * MASK_NEG
    wmlu = np.ascontiguousarray(
        np.asarray(w4mlu, np.float32).reshape(NK, 128).T)
    wc = np.ascontiguousarray(np.asarray(w4C, np.float32).reshape(NK, 128).T)
    wq = np.ascontiguousarray(np.asarray(w4Q, np.float32).reshape(NK, 128).T)
    biascol = np.full((128, 1), float(np.asarray(bias).reshape(-1)[0]), np.float32)

    in_maps = []
    for i in range(NCORES):
        sl = slice(BPC * i, BPC * (i + 1))
        cb, qb = C[sl], Q[sl]
        in_maps.append({
            "CT": np.ascontiguousarray(cb.transpose(0, 2, 1)).reshape(BPC, NK, 128, LC),
            "Cn": cb.reshape(BPC, NJ, 128, D),
            "Qn": qb.reshape(BPC, NM, 128, D),
            "QT": np.ascontiguousarray(qb.transpose(0, 2, 1)).reshape(BPC, NK, 128, LQ),
            "qmadd": np.ascontiguousarray(
                qmadd[sl].reshape(BPC, NM, 128).transpose(0, 2, 1)),
            "cmadd": np.ascontiguousarray(
                cmadd[sl].reshape(BPC, NJ, 128).transpose(0, 2, 1)),
            "wmlu": wmlu, "wc": wc, "wq": wq, "biascol": biascol,
        })
    return in_maps


def run(inputs, trace=False, trace_kwargs=None):
    nc = _get_program()
    in_maps = make_in_maps(**inputs)
    res = run_bass_kernel_spmd(nc, in_maps, list(range(NCORES)),
                               trace=trace, **(trace_kwargs or {}))
    outs = [res.results[i]["out"].reshape(BPC, LC, 4 * D) for i in range(NCORES)]
    full = np.concatenate(outs, axis=0)
    return full, res


def kernel(C, Q, Cmask, Qmask, w4C, w4Q, w4mlu, bias):
    full, _ = run(dict(C=C, Q=Q, Cmask=Cmask, Qmask=Qmask,
                       w4C=w4C, w4Q=w4Q, w4mlu=w4mlu, bias=bias))
    return full


# revision 8
# speedup vs baseline: 1.5005x; 1.5005x over previous
"""CQAttention trilinear-similarity kernel for 8 Trainium2 NeuronCores.

Shapes (full problem): C [16,1024,512], Q [16,256,512] -> out [16,1024,2048].
Sharding: pure batch parallel, 2 batch elements per core, SPMD NEFF.

Math (per batch element), faithful to the reference modulo the max-shift:
  S = (C*w4mlu) @ Q^T + C@w4C + (Q@w4Q)^T + bias          [1024, 256]
  S1 = masked_softmax(S, Qmask, axis=Lq)
  S2 = masked_softmax(S, Cmask, axis=Lc)
  A  = S1 @ Q;  D = S2^T @ C;  Bt = S1 @ D
  out = [C | A | C*A | C*Bt]

Because the reference clips S to [-15,15] *before* exp, exp never overflows,
so the max-subtraction is skipped (error <= 1e-6 relative, dominated by the
reference's own +1e-6 denominator epsilon).  Multiplicative {0,1} masks are
replaced by an additive -60 inside the exp argument (exp(-45) ~ 3e-20 ~ 0),
which lets the mask ride in the scalar-engine activation bias for free.
"""

import os
import sys

for _p in ("/opt/trn_rl_repo", "/root/.axon_site/_ro/trn_rl_repo"):
    if os.path.isdir(_p) and _p not in sys.path:
        sys.path.insert(0, _p)

import numpy as np

import concourse.bacc as bacc
import concourse.mybir as mybir
import concourse.tile as tile
from concourse.bass_utils import run_bass_kernel_spmd
from concourse.masks import make_identity

F32 = mybir.dt.float32
F32R = mybir.dt.float32r
ALU = mybir.AluOpType
ACTF = mybir.ActivationFunctionType

B, LC, LQ, D = 16, 1024, 256, 512
NCORES = 8
BPC = B // NCORES          # batch elements per core
NJ = LC // 128             # 8 Lc tiles
NM = LQ // 128             # 2 Lq tiles
NK = D // 128              # 4 d chunks
MASK_NEG = 60.0
# float32r (~13-bit mantissa, 1 cycle/row) for matmul operands instead of
# float32 (2 half-rate passes).  Measured end-to-end error decides this.
USE_F32R = os.environ.get("KERNEL_F32R", "1") == "1"
MMT = F32R if USE_F32R else F32


def emit(tc, ins, out_d):
    nc = tc.nc
    ct_d = ins["CT"]
    cn_d = ins["Cn"]
    qn_d = ins["Qn"]
    qt_d = ins["QT"]
    qm_d = ins["qmadd"]
    cm_d = ins["cmadd"]
    wmlu_d = ins["wmlu"]
    wc_d = ins["wc"]
    wq_d = ins["wq"]
    bias_d = ins["biascol"]
    if True:
        with tc.tile_pool(name="consts", bufs=1) as consts, \
             tc.tile_pool(name="io", bufs=2) as io, \
             tc.tile_pool(name="io1", bufs=1) as io1, \
             tc.tile_pool(name="mid", bufs=2) as mid, \
             tc.tile_pool(name="mid1", bufs=1) as mid1, \
             tc.tile_pool(name="stgp", bufs=3) as stgp, \
             tc.tile_pool(name="aux", bufs=2) as aux, \
             tc.tile_pool(name="psb", bufs=6, space="PSUM") as psb, \
             tc.tile_pool(name="pss", bufs=2, space="PSUM") as pss:

            ident = consts.tile([128, 128], F32)
            make_identity(nc, ident[:])
            ones_f = consts.tile([128, 2], F32)
            nc.vector.memset(ones_f[:, 0:1], 1.0)
            nc.vector.memset(ones_f[:, 1:2], 0.0)
            onesc = consts.tile([128, 2], MMT)
            nc.vector.tensor_copy(onesc[:], ones_f[:])
            wmlu = consts.tile([128, NK], F32)
            wc = consts.tile([128, NK], F32)
            wq = consts.tile([128, NK], F32)
            biascol = consts.tile([128, 1], F32)
            nc.sync.dma_start(wmlu[:], wmlu_d[:])
            nc.sync.dma_start(wc[:], wc_d[:])
            nc.sync.dma_start(wq[:], wq_d[:])
            nc.sync.dma_start(biascol[:], bias_d[:])
            wq2f = consts.tile([128, NK, 2], F32)
            nc.vector.memset(wq2f[:], 0.0)
            nc.vector.tensor_copy(wq2f[:, :, 0], wq[:])
            wqr = consts.tile([128, NK, 2], MMT)
            nc.vector.tensor_copy(wqr[:], wq2f[:])

            for b in range(BPC):
                # ---- input loads -------------------------------------------------
                ct = io1.tile([128, NK, LC], F32, tag="ct")
                nc.sync.dma_start(ct[:], ct_d[b].rearrange("k p n -> p k n"))
                cn = io.tile([128, NJ, D], F32, tag="cn")
                nc.sync.dma_start(cn[:], cn_d[b].rearrange("j p n -> p j n"))
                qn = io.tile([128, NM, D], F32, tag="qn")
                nc.sync.dma_start(qn[:], qn_d[b].rearrange("m p n -> p m n"))
                qt = io.tile([128, NK, LQ], F32, tag="qt")
                nc.sync.dma_start(qt[:], qt_d[b].rearrange("k p n -> p k n"))
                if USE_F32R:
                    ctr = io1.tile([128, NK, LC], MMT, tag="ctr")
                    nc.vector.tensor_copy(ctr[:], ct[:])
                    qnr = mid1.tile([128, NM, D], MMT, tag="qnr")
                    nc.vector.tensor_copy(qnr[:], qn[:])
                    cnr = mid1.tile([128, NJ, D], MMT, tag="cnr")
                    nc.vector.tensor_copy(cnr[:], cn[:])
                    qtr = mid1.tile([128, NK, LQ], MMT, tag="qtr")
                    nc.vector.tensor_copy(qtr[:], qt[:])
                else:
                    ctr, qnr, cnr, qtr = ct, qn, cn, qt
                qm = io.tile([128, NM], F32, tag="qm")
                nc.sync.dma_start(qm[:], qm_d[b])
                cm = io.tile([128, NJ], F32, tag="cm")
                nc.sync.dma_start(cm[:], cm_d[b])

                # ---- QTm = QT * w4mlu + w4C  (folds sub2 scaling and sub0) ------
                qtm = mid1.tile([128, NK, LQ], MMT, tag="qtm")
                for k in range(NK):
                    nc.vector.tensor_scalar(qtm[:, k], qt[:, k],
                                            wmlu[:, k:k + 1], wc[:, k:k + 1],
                                            ALU.mult, ALU.add)

                # ---- sub1 + bias, per Lq tile -----------------------------------
                s1b = mid.tile([128, NM], F32, tag="s1b")
                for m in range(NM):
                    ps_sub1 = pss.tile([128, 2], F32, tag="psmall")
                    for k in range(NK):
                        nc.tensor.matmul(ps_sub1[:], qtr[:, k, m * 128:(m + 1) * 128],
                                         wqr[:, k],
                                         start=(k == 0), stop=(k == NK - 1))
                    nc.vector.tensor_tensor(s1b[:, m:m + 1], ps_sub1[:, 0:1],
                                            biascol[:], ALU.add)

                # ---- S^T matmuls + clip chain + e1 = exp masked -----------------
                xc = []
                for m in range(NM):
                    x = mid1.tile([128, LC], F32, tag=f"xc{m}")
                    for n in range(2):
                        ps_st = psb.tile([128, 512], F32, tag="pbig")
                        for k in range(NK):
                            nc.tensor.matmul(
                                ps_st[:],
                                qtm[:, k, m * 128:(m + 1) * 128],
                                ctr[:, k, n * 512:(n + 1) * 512],
                                start=(k == 0), stop=(k == NK - 1))
                        # x = min(S^T + sub1 + bias, 15)
                        nc.vector.tensor_scalar(x[:, n * 512:(n + 1) * 512], ps_st[:],
                                                s1b[:, m:m + 1], 15.0,
                                                ALU.add, ALU.min)
                    nc.vector.tensor_scalar_max(x[:], x[:], -15.0)
                    xc.append(x)

                e1 = []
                for m in range(NM):
                    e = mid1.tile([128, LC], MMT, tag=f"e1{m}")
                    nc.scalar.activation(e[:], xc[m][:], ACTF.Exp,
                                         bias=qm[:, m:m + 1], scale=1.0)
                    e1.append(e)

                # ---- transpose x -> natural layout, e2 = exp masked -------------
                e2 = mid1.tile([128, NJ, LQ], MMT, tag="e2")
                for p in range(NJ // 2):
                    ps_xt = psb.tile([128, 2 * LQ], F32, tag="pbig")
                    for jj in range(2):
                        j = 2 * p + jj
                        for m in range(NM):
                            nc.tensor.transpose(
                                ps_xt[:, jj * LQ + m * 128: jj * LQ + (m + 1) * 128],
                                xc[m][:, j * 128:(j + 1) * 128], ident[:])
                    for jj in range(2):
                        j = 2 * p + jj
                        nc.scalar.activation(e2[:, j], ps_xt[:, jj * LQ:(jj + 1) * LQ],
                                             ACTF.Exp, bias=cm[:, j:j + 1], scale=1.0)

                # ---- s2 column sums -> r2 ---------------------------------------
                s2s = mid.tile([128, NM], F32, tag="s2s")
                for m in range(NM):
                    ps_s2 = pss.tile([128, 2], F32, tag="psmall")
                    for j in range(NJ):
                        nc.tensor.matmul(ps_s2[:], e2[:, j, m * 128:(m + 1) * 128],
                                         onesc[:],
                                         start=(j == 0), stop=(j == NJ - 1))
                    nc.vector.tensor_scalar_add(s2s[:, m:m + 1], ps_s2[:, 0:1], 1e-6)
                r2 = mid.tile([128, NM], F32, tag="r2")
                nc.vector.reciprocal(r2[:], s2s[:])

                # ---- D = diag(r2) (e2^T @ C) ------------------------------------
                dD = mid1.tile([128, NM, D], MMT, tag="dD")
                for m in range(NM):
                    ps_d = psb.tile([128, D], F32, tag="pbig")
                    for j in range(NJ):
                        nc.tensor.matmul(ps_d[:], e2[:, j, m * 128:(m + 1) * 128],
                                         cnr[:, j], start=(j == 0), stop=(j == NJ - 1))
                    nc.scalar.mul(dD[:, m], ps_d[:], r2[:, m:m + 1])

                # ---- A_raw, Bt_raw, s1 sums; normalize + combine + store --------
                for j in range(NJ):
                    ps_a = psb.tile([128, D], F32, tag="pbig")
                    ps_bt = psb.tile([128, D], F32, tag="pbig")
                    ps_s1 = pss.tile([128, 2], F32, tag="psmall")
                    for m in range(NM):
                        lhs = e1[m][:, j * 128:(j + 1) * 128]
                        nc.tensor.matmul(ps_a[:], lhs, qnr[:, m],
                                         start=(m == 0), stop=(m == NM - 1))
                        nc.tensor.matmul(ps_bt[:], lhs, dD[:, m],
                                         start=(m == 0), stop=(m == NM - 1))
                        nc.tensor.matmul(ps_s1[:], lhs, onesc[:],
                                         start=(m == 0), stop=(m == NM - 1))
                    r1 = aux.tile([128, 1], F32, tag="r1")
                    nc.vector.tensor_scalar_add(r1[:], ps_s1[:, 0:1], 1e-6)
                    nc.vector.reciprocal(r1[:], r1[:])

                    stg = stgp.tile([128, 3 * D], F32, tag="stg")
                    # A = A_raw * r1
                    nc.scalar.mul(stg[:, 0:D], ps_a[:], r1[:])
                    # C*A = (A_raw * r1) * C
                    nc.vector.scalar_tensor_tensor(stg[:, D:2 * D], ps_a[:], r1[:],
                                                   cn[:, j], ALU.mult, ALU.mult)
                    # Bt = Bt_raw * r1 (scratch), then C*Bt on gpsimd
                    btn = aux.tile([128, D], F32, tag="btn")
                    nc.scalar.mul(btn[:], ps_bt[:], r1[:])
                    nc.gpsimd.tensor_mul(stg[:, 2 * D:3 * D], btn[:], cn[:, j])

                    nc.scalar.dma_start(out_d[b, j][:, 0:D], cn[:, j])
                    nc.scalar.dma_start(out_d[b, j][:, D:4 * D], stg[:])


def build_program():
    nc = bacc.Bacc("TRN2", target_bir_lowering=False, debug=False,
                   enable_asserts=False, num_devices=NCORES)
    names_shapes = [
        ("CT", [BPC, NK, 128, LC]), ("Cn", [BPC, NJ, 128, D]),
        ("Qn", [BPC, NM, 128, D]), ("QT", [BPC, NK, 128, LQ]),
        ("qmadd", [BPC, 128, NM]), ("cmadd", [BPC, 128, NJ]),
        ("wmlu", [128, NK]), ("wc", [128, NK]), ("wq", [128, NK]),
        ("biascol", [128, 1]),
    ]
    ins = {n: nc.declare_dram_parameter(n, sh, F32, isOutput=False)
           for n, sh in names_shapes}
    out_d = nc.declare_dram_parameter("out", [BPC, NJ, 128, 4 * D], F32,
                                      isOutput=True)
    with tile.TileContext(nc) as tc:
        emit(tc, ins, out_d)
    nc.compile()
    return nc


_NC_CACHE = None


def _get_program():
    global _NC_CACHE
    if _NC_CACHE is None:
        _NC_CACHE = build_program()
    return _NC_CACHE


def make_in_maps(C, Q, Cmask, Qmask, w4C, w4Q, w4mlu, bias):
    C = np.ascontiguousarray(C, dtype=np.float32)
    Q = np.ascontiguousarray(Q, dtype=np.float32)
    qmadd = (Qmask.astype(np.float32) - 1.0) * MASK_NEG
    cmadd = (Cmask.astype(np.float32) - 1.0) * MASK_NEG
    wmlu = np.ascontiguousarray(
        np.asarray(w4mlu, np.float32).reshape(NK, 128).T)
    wc = np.ascontiguousarray(np.asarray(w4C, np.float32).reshape(NK, 128).T)
    wq = np.ascontiguousarray(np.asarray(w4Q, np.float32).reshape(NK, 128).T)
    biascol = np.full((128, 1), float(np.asarray(bias).reshape(-1)[0]), np.float32)

    in_maps = []
    for i in range(NCORES):
        sl = slice(BPC * i, BPC * (i + 1))
        cb, qb = C[sl], Q[sl]
        in_maps.append({
            "CT": np.ascontiguousarray(cb.transpose(0, 2, 1)).reshape(BPC, NK, 128, LC),
            "Cn": cb.reshape(BPC, NJ, 128, D),
            "Qn": qb.reshape(BPC, NM, 128, D),
            "QT": np.ascontiguousarray(qb.transpose(0, 2, 1)).reshape(BPC, NK, 128, LQ),
            "qmadd": np.ascontiguousarray(
                qmadd[sl].reshape(BPC, NM, 128).transpose(0, 2, 1)),
            "cmadd": np.ascontiguousarray(
                cmadd[sl].reshape(BPC, NJ, 128).transpose(0, 2, 1)),
            "wmlu": wmlu, "wc": wc, "wq": wq, "biascol": biascol,
        })
    return in_maps


def run(inputs, trace=False, trace_kwargs=None):
    nc = _get_program()
    in_maps = make_in_maps(**inputs)
    res = run_bass_kernel_spmd(nc, in_maps, list(range(NCORES)),
                               trace=trace, **(trace_kwargs or {}))
    outs = [res.results[i]["out"].reshape(BPC, LC, 4 * D) for i in range(NCORES)]
    full = np.concatenate(outs, axis=0)
    return full, res


def kernel(C, Q, Cmask, Qmask, w4C, w4Q, w4mlu, bias):
    full, _ = run(dict(C=C, Q=Q, Cmask=Cmask, Qmask=Qmask,
                       w4C=w4C, w4Q=w4Q, w4mlu=w4mlu, bias=bias))
    return full


# revision 9
# speedup vs baseline: 1.5075x; 1.0046x over previous
"""CQAttention trilinear-similarity kernel for 8 Trainium2 NeuronCores.

Shapes (full problem): C [16,1024,512], Q [16,256,512] -> out [16,1024,2048].
Sharding: pure batch parallel, 2 batch elements per core, SPMD NEFF.

Math (per batch element), faithful to the reference modulo the max-shift:
  S = (C*w4mlu) @ Q^T + C@w4C + (Q@w4Q)^T + bias          [1024, 256]
  S1 = masked_softmax(S, Qmask, axis=Lq)
  S2 = masked_softmax(S, Cmask, axis=Lc)
  A  = S1 @ Q;  D = S2^T @ C;  Bt = S1 @ D
  out = [C | A | C*A | C*Bt]

Because the reference clips S to [-15,15] *before* exp, exp never overflows,
so the max-subtraction is skipped (error <= 1e-6 relative, dominated by the
reference's own +1e-6 denominator epsilon).  Multiplicative {0,1} masks are
replaced by an additive -60 inside the exp argument (exp(-45) ~ 3e-20 ~ 0),
which lets the mask ride in the scalar-engine activation bias for free.
"""

import os
import sys

for _p in ("/opt/trn_rl_repo", "/root/.axon_site/_ro/trn_rl_repo"):
    if os.path.isdir(_p) and _p not in sys.path:
        sys.path.insert(0, _p)

import numpy as np

import concourse.bacc as bacc
import concourse.mybir as mybir
import concourse.tile as tile
from concourse.bass_utils import run_bass_kernel_spmd
from concourse.masks import make_identity

F32 = mybir.dt.float32
F32R = mybir.dt.float32r
ALU = mybir.AluOpType
ACTF = mybir.ActivationFunctionType

B, LC, LQ, D = 16, 1024, 256, 512
NCORES = 8
BPC = B // NCORES          # batch elements per core
NJ = LC // 128             # 8 Lc tiles
NM = LQ // 128             # 2 Lq tiles
NK = D // 128              # 4 d chunks
MASK_NEG = 60.0
# float32r (~13-bit mantissa, 1 cycle/row) for matmul operands instead of
# float32 (2 half-rate passes).  Measured end-to-end error decides this.
USE_F32R = os.environ.get("KERNEL_F32R", "1") == "1"
MMT = F32R if USE_F32R else F32


def emit(tc, ins, out_d):
    nc = tc.nc
    ct_d = ins["CT"]
    cn_d = ins["Cn"]
    qn_d = ins["Qn"]
    qt_d = ins["QT"]
    qm_d = ins["qmadd"]
    cm_d = ins["cmadd"]
    wmlu_d = ins["wmlu"]
    wc_d = ins["wc"]
    wq_d = ins["wq"]
    bias_d = ins["biascol"]
    if True:
        with tc.tile_pool(name="consts", bufs=1) as consts, \
             tc.tile_pool(name="io", bufs=2) as io, \
             tc.tile_pool(name="io1", bufs=1) as io1, \
             tc.tile_pool(name="mid", bufs=2) as mid, \
             tc.tile_pool(name="mid1", bufs=1) as mid1, \
             tc.tile_pool(name="stgp", bufs=3) as stgp, \
             tc.tile_pool(name="aux", bufs=2) as aux, \
             tc.tile_pool(name="psb", bufs=6, space="PSUM") as psb, \
             tc.tile_pool(name="pss", bufs=2, space="PSUM") as pss:

            ident = consts.tile([128, 128], F32)
            make_identity(nc, ident[:])
            ones_f = consts.tile([128, 2], F32)
            nc.vector.memset(ones_f[:, 0:1], 1.0)
            nc.vector.memset(ones_f[:, 1:2], 0.0)
            onesc = consts.tile([128, 2], MMT)
            nc.vector.tensor_copy(onesc[:], ones_f[:])
            wmlu = consts.tile([128, NK], F32)
            wc = consts.tile([128, NK], F32)
            wq = consts.tile([128, NK], F32)
            biascol = consts.tile([128, 1], F32)
            nc.sync.dma_start(wmlu[:], wmlu_d[:])
            nc.sync.dma_start(wc[:], wc_d[:])
            nc.sync.dma_start(wq[:], wq_d[:])
            nc.sync.dma_start(biascol[:], bias_d[:])
            wq2f = consts.tile([128, NK, 2], F32)
            nc.vector.memset(wq2f[:], 0.0)
            nc.vector.tensor_copy(wq2f[:, :, 0], wq[:])
            wqr = consts.tile([128, NK, 2], MMT)
            nc.vector.tensor_copy(wqr[:], wq2f[:])

            for b in range(BPC):
                # ---- input loads -------------------------------------------------
                ctr = io.tile([128, NK, LC], MMT, tag="ctr")
                nc.sync.dma_start(ctr[:], ct_d[b].rearrange("k p n -> p k n").bitcast(MMT))
                cn = io.tile([128, NJ, D], F32, tag="cn")
                nc.sync.dma_start(cn[:], cn_d[b].rearrange("j p n -> p j n"))
                qn = io.tile([128, NM, D], F32, tag="qn")
                nc.sync.dma_start(qn[:], qn_d[b].rearrange("m p n -> p m n"))
                qtr = io.tile([128, NK, LQ], MMT, tag="qtr")
                nc.sync.dma_start(qtr[:], qt_d[b].rearrange("k p n -> p k n").bitcast(MMT))
                if USE_F32R:
                    qnr = mid1.tile([128, NM, D], MMT, tag="qnr")
                    nc.vector.tensor_copy(qnr[:], qn[:])
                    cnr = mid1.tile([128, NJ, D], MMT, tag="cnr")
                    nc.vector.tensor_copy(cnr[:], cn[:])
                else:
                    qnr, cnr = qn, cn
                qm = io.tile([128, NM], F32, tag="qm")
                nc.sync.dma_start(qm[:], qm_d[b])
                cm = io.tile([128, NJ], F32, tag="cm")
                nc.sync.dma_start(cm[:], cm_d[b])

                # ---- QTm = QT * w4mlu + w4C  (folds sub2 scaling and sub0) ------
                qtm = mid.tile([128, NK, LQ], MMT, tag="qtm")
                for k in range(NK):
                    nc.vector.tensor_scalar(qtm[:, k], qtr[:, k].bitcast(F32),
                                            wmlu[:, k:k + 1], wc[:, k:k + 1],
                                            ALU.mult, ALU.add)

                # ---- sub1 + bias, per Lq tile -----------------------------------
                s1b = mid.tile([128, NM], F32, tag="s1b")
                for m in range(NM):
                    ps_sub1 = pss.tile([128, 2], F32, tag="psmall")
                    for k in range(NK):
                        nc.tensor.matmul(ps_sub1[:], qtr[:, k, m * 128:(m + 1) * 128],
                                         wqr[:, k],
                                         start=(k == 0), stop=(k == NK - 1))
                    nc.vector.tensor_tensor(s1b[:, m:m + 1], ps_sub1[:, 0:1],
                                            biascol[:], ALU.add)

                # ---- S^T matmuls + clip chain + e1 = exp masked -----------------
                xc = []
                for m in range(NM):
                    x = mid1.tile([128, LC], F32, tag=f"xc{m}")
                    for n in range(2):
                        ps_st = psb.tile([128, 512], F32, tag="pbig")
                        for k in range(NK):
                            nc.tensor.matmul(
                                ps_st[:],
                                qtm[:, k, m * 128:(m + 1) * 128],
                                ctr[:, k, n * 512:(n + 1) * 512],
                                start=(k == 0), stop=(k == NK - 1))
                        # x = min(S^T + sub1 + bias, 15)
                        nc.vector.tensor_scalar(x[:, n * 512:(n + 1) * 512], ps_st[:],
                                                s1b[:, m:m + 1], 15.0,
                                                ALU.add, ALU.min)
                    nc.vector.tensor_scalar_max(x[:], x[:], -15.0)
                    xc.append(x)

                e1 = []
                for m in range(NM):
                    e = mid.tile([128, LC], MMT, tag=f"e1{m}")
                    nc.scalar.activation(e[:], xc[m][:], ACTF.Exp,
                                         bias=qm[:, m:m + 1], scale=1.0)
                    e1.append(e)

                # ---- transpose x -> natural layout, e2 = exp masked -------------
                e2 = mid.tile([128, NJ, LQ], MMT, tag="e2")
                for p in range(NJ // 2):
                    ps_xt = psb.tile([128, 2 * LQ], F32, tag="pbig")
                    for jj in range(2):
                        j = 2 * p + jj
                        for m in range(NM):
                            nc.tensor.transpose(
                                ps_xt[:, jj * LQ + m * 128: jj * LQ + (m + 1) * 128],
                                xc[m][:, j * 128:(j + 1) * 128], ident[:])
                    for jj in range(2):
                        j = 2 * p + jj
                        nc.scalar.activation(e2[:, j], ps_xt[:, jj * LQ:(jj + 1) * LQ],
                                             ACTF.Exp, bias=cm[:, j:j + 1], scale=1.0)

                # ---- s2 column sums -> r2 ---------------------------------------
                s2s = mid.tile([128, NM], F32, tag="s2s")
                for m in range(NM):
                    ps_s2 = pss.tile([128, 2], F32, tag="psmall")
                    for j in range(NJ):
                        nc.tensor.matmul(ps_s2[:], e2[:, j, m * 128:(m + 1) * 128],
                                         onesc[:],
                                         start=(j == 0), stop=(j == NJ - 1))
                    nc.vector.tensor_scalar_add(s2s[:, m:m + 1], ps_s2[:, 0:1], 1e-6)
                r2 = mid.tile([128, NM], F32, tag="r2")
                nc.vector.reciprocal(r2[:], s2s[:])

                # ---- D = diag(r2) (e2^T @ C) ------------------------------------
                dD = mid.tile([128, NM, D], MMT, tag="dD")
                for m in range(NM):
                    ps_d = psb.tile([128, D], F32, tag="pbig")
                    for j in range(NJ):
                        nc.tensor.matmul(ps_d[:], e2[:, j, m * 128:(m + 1) * 128],
                                         cnr[:, j], start=(j == 0), stop=(j == NJ - 1))
                    nc.scalar.mul(dD[:, m], ps_d[:], r2[:, m:m + 1])

                # ---- A_raw, Bt_raw, s1 sums; normalize + combine + store --------
                for j in range(NJ):
                    ps_a = psb.tile([128, D], F32, tag="pbig")
                    ps_bt = psb.tile([128, D], F32, tag="pbig")
                    ps_s1 = pss.tile([128, 2], F32, tag="psmall")
                    for m in range(NM):
                        lhs = e1[m][:, j * 128:(j + 1) * 128]
                        nc.tensor.matmul(ps_a[:], lhs, qnr[:, m],
                                         start=(m == 0), stop=(m == NM - 1))
                        nc.tensor.matmul(ps_bt[:], lhs, dD[:, m],
                                         start=(m == 0), stop=(m == NM - 1))
                        nc.tensor.matmul(ps_s1[:], lhs, onesc[:],
                                         start=(m == 0), stop=(m == NM - 1))
                    r1 = aux.tile([128, 1], F32, tag="r1")
                    nc.vector.tensor_scalar_add(r1[:], ps_s1[:, 0:1], 1e-6)
                    nc.vector.reciprocal(r1[:], r1[:])

                    stg = stgp.tile([128, 3 * D], F32, tag="stg")
                    # A = A_raw * r1
                    nc.scalar.mul(stg[:, 0:D], ps_a[:], r1[:])
                    # C*A = (A_raw * r1) * C
                    nc.vector.scalar_tensor_tensor(stg[:, D:2 * D], ps_a[:], r1[:],
                                                   cn[:, j], ALU.mult, ALU.mult)
                    # Bt = Bt_raw * r1 (scratch), then C*Bt on gpsimd
                    btn = aux.tile([128, D], F32, tag="btn")
                    nc.scalar.mul(btn[:], ps_bt[:], r1[:])
                    nc.gpsimd.tensor_mul(stg[:, 2 * D:3 * D], btn[:], cn[:, j])

                    nc.scalar.dma_start(out_d[b, j][:, 0:D], cn[:, j])
                    nc.scalar.dma_start(out_d[b, j][:, D:4 * D], stg[:])


def build_program():
    nc = bacc.Bacc("TRN2", target_bir_lowering=False, debug=False,
                   enable_asserts=False, num_devices=NCORES)
    names_shapes = [
        ("CT", [BPC, NK, 128, LC]), ("Cn", [BPC, NJ, 128, D]),
        ("Qn", [BPC, NM, 128, D]), ("QT", [BPC, NK, 128, LQ]),
        ("qmadd", [BPC, 128, NM]), ("cmadd", [BPC, 128, NJ]),
        ("wmlu", [128, NK]), ("wc", [128, NK]), ("wq", [128, NK]),
        ("biascol", [128, 1]),
    ]
    ins = {n: nc.declare_dram_parameter(n, sh, F32, isOutput=False)
           for n, sh in names_shapes}
    out_d = nc.declare_dram_parameter("out", [BPC, NJ, 128, 4 * D], F32,
                                      isOutput=True)
    with tile.TileContext(nc) as tc:
        emit(tc, ins, out_d)
    nc.compile()
    return nc


_NC_CACHE = None


def _get_program():
    global _NC_CACHE
    if _NC_CACHE is None:
        _NC_CACHE = build_program()
    return _NC_CACHE


def _round_f32r(a):
    u = np.ascontiguousarray(a, np.float32).view(np.uint32).astype(np.uint64)
    u = (u + 0x800) & 0xFFFFF000
    return u.astype(np.uint32).view(np.float32)


def make_in_maps(C, Q, Cmask, Qmask, w4C, w4Q, w4mlu, bias):
    C = np.ascontiguousarray(C, dtype=np.float32)
    Q = np.ascontiguousarray(Q, dtype=np.float32)
    qmadd = (Qmask.astype(np.float32) - 1.0) * MASK_NEG
    cmadd = (Cmask.astype(np.float32) - 1.0) * MASK_NEG
    wmlu = np.ascontiguousarray(
        np.asarray(w4mlu, np.float32).reshape(NK, 128).T)
    wc = np.ascontiguousarray(np.asarray(w4C, np.float32).reshape(NK, 128).T)
    wq = np.ascontiguousarray(np.asarray(w4Q, np.float32).reshape(NK, 128).T)
    biascol = np.full((128, 1), float(np.asarray(bias).reshape(-1)[0]), np.float32)

    in_maps = []
    for i in range(NCORES):
        sl = slice(BPC * i, BPC * (i + 1))
        cb, qb = C[sl], Q[sl]
        in_maps.append({
            "CT": (_round_f32r if USE_F32R else np.ascontiguousarray)(
                np.ascontiguousarray(cb.transpose(0, 2, 1))).reshape(BPC, NK, 128, LC),
            "Cn": cb.reshape(BPC, NJ, 128, D),
            "Qn": qb.reshape(BPC, NM, 128, D),
            "QT": (_round_f32r if USE_F32R else np.ascontiguousarray)(
                np.ascontiguousarray(qb.transpose(0, 2, 1))).reshape(BPC, NK, 128, LQ),
            "qmadd": np.ascontiguousarray(
                qmadd[sl].reshape(BPC, NM, 128).transpose(0, 2, 1)),
            "cmadd": np.ascontiguousarray(
                cmadd[sl].reshape(BPC, NJ, 128).transpose(0, 2, 1)),
            "wmlu": wmlu, "wc": wc, "wq": wq, "biascol": biascol,
        })
    return in_maps


def run(inputs, trace=False, trace_kwargs=None):
    nc = _get_program()
    in_maps = make_in_maps(**inputs)
    res = run_bass_kernel_spmd(nc, in_maps, list(range(NCORES)),
                               trace=trace, **(trace_kwargs or {}))
    outs = [res.results[i]["out"].reshape(BPC, LC, 4 * D) for i in range(NCORES)]
    full = np.concatenate(outs, axis=0)
    return full, res


def kernel(C, Q, Cmask, Qmask, w4C, w4Q, w4mlu, bias):
    full, _ = run(dict(C=C, Q=Q, Cmask=Cmask, Qmask=Qmask,
                       w4C=w4C, w4Q=w4Q, w4mlu=w4mlu, bias=bias))
    return full
